# revision 20
# baseline (speedup 1.0000x reference)
"""Chamfer distance kernel for Trainium2 (8 NeuronCores, data-parallel over batch).

Problem: xyz1, xyz2: [8, 8192, 3] fp32.  Per batch b:
  d[i,j] = |x_i|^2 + |y_j|^2 - 2 x_i.y_j
  dist1[i] = min_j d[i,j]; idx1[i] = argmin_j d[i,j]   (and symmetrically dist2/idx2)

Strategy (one batch per core):
  - PE computes NEGATED distances e = 2 x.y - sq1 - sq2 with a K=5 matmul:
      lhsT rows: [x0, x1, x2, 1, -sq1]   (stationary, per 128-row tile)
      rhs  rows: [2y0, 2y1, 2y2, -sq2, 1] (moving, 512-column chunks)
    so min/argmin of d becomes max/argmax of e.
  - Row path: ACT copies each PSUM chunk into a full-row SBUF buffer
    E [128, 8192]; DVE `max` (top-8) + `max_index` give row max + argmax.
  - Col path: DVE running tensor_max into ACC [128, 8192] plus a
    copy_predicated tracker IT of the first row-tile t achieving the max;
    GPSIMD partition_all_reduce(max) finishes across partitions, with
    index tie-break resolved by maximizing -(128 t + p).
"""

import os
import numpy as np

B = 8
N = 8192  # rows per batch (xyz1 points)
M = 8192  # cols per batch (xyz2 points)
P = 128
CHUNK = 512
N_CORES = 8

_cache = {}


def build(n=N, m=M, n_cores=N_CORES):
    """Build the Bass program. Returns the compiled Bacc object."""
    import concourse.bacc as bacc
    import concourse.tile as tile
    import concourse.mybir as mybir
    from concourse.bass_isa import ReduceOp

    dt = mybir.dt
    Alu = mybir.AluOpType
    Act = mybir.ActivationFunctionType

    nt = n // P        # row tiles
    nch = m // CHUNK   # column chunks
    fin = 2048 if m % 2048 == 0 else m  # finals chunk width
    nq = m // fin

    nc = bacc.Bacc(
        "TRN2",
        target_bir_lowering=False,
        debug=False,
        enable_asserts=False,
        num_devices=n_cores,
    )

    xt = nc.dram_tensor("xt", [3, n], dt.float32, kind="ExternalInput").ap()
    yt = nc.dram_tensor("yt", [3, m], dt.float32, kind="ExternalInput").ap()
    idx1_d = nc.dram_tensor("idx1", [P, nt], dt.uint32, kind="ExternalOutput").ap()
    idx2_d = nc.dram_tensor("idx2", [1, m], dt.uint32, kind="ExternalOutput").ap()

    with tile.TileContext(nc) as tc, tc.tile_pool(name="persist", bufs=1) as pp:
        # ---- constants / prep ----
        lhs = pp.tile([5, n], dt.float32, name="lhs")
        rhs = pp.tile([5, m], dt.float32, name="rhs")
        ones3 = pp.tile([3, 1], dt.float32, name="ones3")
        piota_u = pp.tile([P, 1], dt.uint32, name="piota_u")
        npiota_f = pp.tile([P, 1], dt.float32, name="npiota_f")

        nc.vector.memset(ones3[:], 1.0)
        # piota_u[p, 0] = p ; npiota_f = -p
        nc.gpsimd.iota(piota_u[:], pattern=[[0, 1]], base=0, channel_multiplier=1)
        nc.vector.tensor_scalar_mul(npiota_f[:], piota_u[:], -1.0)

        # load points
        nc.sync.dma_start(lhs[0:3, :], xt[:, :])
        nc.sync.dma_start(rhs[0:3, :], yt[:, :])

        # squared norms via ones^T @ (pts^2).  Engine ops must start at
        # partition 0, so -sq goes to a base-0 scratch row, then SBUF->SBUF
        # DMA places it into partition 3/4 of lhs/rhs.
        with (
            tc.tile_pool(name="sq_pool", bufs=1) as sqp,
            tc.tile_pool(name="psum_sq", bufs=2, space="PSUM") as psum_sq,
        ):
            sq_tmp = sqp.tile([3, max(n, m)], dt.float32, name="sq_tmp")
            # one base-0 scratch row, reused sequentially: ones -> -sq1 -> -sq2
            row = sqp.tile([1, max(n, m)], dt.float32, name="row")
            nc.vector.memset(row[:], 1.0)
            nc.sync.dma_start(lhs[3:4, :], row[:, 0:n])
            nc.sync.dma_start(rhs[4:5, :], row[:, 0:m])
            nsq1 = row[:, 0:n]
            nsq2 = row[:, 0:m]

            wn = min(CHUNK, n)
            nc.scalar.activation(sq_tmp[:, 0:n], lhs[0:3, :], Act.Square)
            for c in range(n // wn):
                ps = psum_sq.tile([1, wn], dt.float32, tag="ps_sq")
                nc.tensor.matmul(
                    ps[:], ones3[:], sq_tmp[:, c * wn:(c + 1) * wn],
                    start=True, stop=True,
                )
                nc.scalar.activation(
                    nsq1[:, c * wn:(c + 1) * wn], ps[:], Act.Copy, scale=-1.0
                )
            nc.sync.dma_start(lhs[4:5, :], nsq1[:])
            wm = min(CHUNK, m)
            nc.scalar.activation(sq_tmp[:, 0:m], rhs[0:3, :], Act.Square)
            for c in range(m // wm):
                ps = psum_sq.tile([1, wm], dt.float32, tag="ps_sq")
                nc.tensor.matmul(
                    ps[:], ones3[:], sq_tmp[:, c * wm:(c + 1) * wm],
                    start=True, stop=True,
                )
                nc.scalar.activation(
                    nsq2[:, c * wm:(c + 1) * wm], ps[:], Act.Copy, scale=-1.0
                )
            # lhs rows: [x0 x1 x2, 1, -sq1]; rhs rows: [2y0 2y1 2y2, -sq2, 1]
            nc.sync.dma_start(rhs[3:4, :], nsq2[:])
        # scale y by 2 (after sq2 computed)
        nc.vector.tensor_scalar_mul(rhs[0:3, :], rhs[0:3, :], 2.0)

        # ---- persistent state ----
        acc = pp.tile([P, m], dt.float16, name="acc")      # running col max of e (fp16)
        it_t = pp.tile([P, m], dt.float16, name="it_t")    # first row-tile idx achieving acc
        idxr = pp.tile([P, nt], dt.uint32, name="idxr")

        nc.vector.memset(acc[:], -60000.0)
        nc.vector.memset(it_t[:], 0)

        # ---- main loop ----
        with (
            tc.tile_pool(name="psum_e", bufs=6, space="PSUM") as psum_e,
            tc.tile_pool(name="e_pool", bufs=2) as e_pool,
            tc.tile_pool(name="cmp_pool", bufs=4) as cmp_pool,
            tc.tile_pool(name="top8", bufs=2) as top8_pool,
        ):
            for t in range(nt):
                e_row = e_pool.tile([P, m], dt.float16, tag="e_row")
                for c in range(nch):
                    cs = slice(c * CHUNK, (c + 1) * CHUNK)
                    ps = psum_e.tile([P, CHUNK], dt.float32, tag="ps")
                    nc.tensor.matmul(
                        ps[:], lhs[:, t * P:(t + 1) * P], rhs[:, cs],
                        start=True, stop=True,
                    )
                    # row path raw data (SBUF copy)
                    nc.scalar.copy(e_row[:, cs], ps[:])
                    # col path (all fp16 SBUF, 2x DVE mode): update running
                    # max, detect "this tile won" via e >= acc', record t.
                    # t strictly increases, so it = max(it, t*cmp) keeps the
                    # winning row-tile.
                    nc.vector.tensor_max(acc[:, cs], acc[:, cs], e_row[:, cs])
                    if t > 0:
                        cmp = cmp_pool.tile([P, CHUNK], dt.float16, tag="cmp")
                        nc.vector.tensor_tensor(cmp[:], e_row[:, cs], acc[:, cs], op=Alu.is_ge)
                        nc.vector.scalar_tensor_tensor(
                            it_t[:, cs], cmp[:], float(t), it_t[:, cs],
                            op0=Alu.mult, op1=Alu.max,
                        )
                # row path: top-8 + index of max
                max8 = top8_pool.tile([P, 8], dt.float16, tag="max8")
                idx8 = top8_pool.tile([P, 8], dt.uint32, tag="idx8")
                nc.vector.max(max8[:], e_row[:])
                nc.vector.max_index(idx8[:], max8[:], e_row[:])
                nc.vector.tensor_copy(idxr[:, t:t + 1], idx8[:, 0:1])

        # ---- row outputs ----
        nc.sync.dma_start(idx1_d[:, :], idxr[:])

        # ---- col outputs (chunked finals) ----
        with tc.tile_pool(name="fin_pool", bufs=1) as fp:
            for q in range(nq):
                qs = slice(q * fin, (q + 1) * fin)
                ar = fp.tile([P, fin], dt.float16, tag="ar")
                nc.gpsimd.partition_all_reduce(ar[:], acc[:, qs], P, ReduceOp.max)
                mq = fp.tile([P, fin], dt.uint8, tag="mq")
                nc.vector.tensor_tensor(mq[:], acc[:, qs], ar[:], op=Alu.is_equal)
                # ng = -(128*t + p) for candidates
                ng = fp.tile([P, fin], dt.float32, tag="ng")
                nc.vector.tensor_scalar(
                    ng[:], it_t[:, qs], -128.0, npiota_f[:],
                    op0=Alu.mult, op1=Alu.add,
                )
                sel = fp.tile([P, fin], dt.float32, tag="sel")
                nc.vector.memset(sel[:], -1e30)
                nc.vector.copy_predicated(sel[:], mq[:], ng[:])
                ar2 = fp.tile([P, fin], dt.float32, tag="ar2")
                nc.gpsimd.partition_all_reduce(ar2[:], sel[:], P, ReduceOp.max)
                # idx2 = -ar2
                i2f = fp.tile([1, fin], dt.float32, tag="i2f")
                nc.scalar.activation(i2f[:], ar2[0:1, :], Act.Copy, scale=-1.0)
                i2u = fp.tile([1, fin], dt.uint32, tag="i2u")
                nc.vector.tensor_copy(i2u[:], i2f[:])
                nc.sync.dma_start(idx2_d[:, qs], i2u[:])

    nc.compile()
    return nc


def _run(nc, xyz1, xyz2, n_cores, trace=False):
    from concourse import bass_utils

    in_maps = []
    for b in range(n_cores):
        in_maps.append({
            "xt": np.ascontiguousarray(xyz1[b].T).astype(np.float32),
            "yt": np.ascontiguousarray(xyz2[b].T).astype(np.float32),
        })
    res = bass_utils.run_bass_kernel_spmd(
        nc, in_maps, core_ids=list(range(n_cores)), trace=trace,
    )
    return res


def _host_dists(x, y, idx1, idx2):
    """Exact fp32 dists from device indices, same formula as the reference:
    d = sq1 + sq2 - 2 x.y  (device only finds argmins)."""
    sq1 = (x * x).sum(-1)                      # [n]
    sq2 = (y * y).sum(-1)                      # [m]
    g1 = y[idx1]                               # [n, 3]
    dist1 = sq1 + sq2[idx1] - 2.0 * (x * g1).sum(-1)
    g2 = x[idx2]                               # [m, 3]
    dist2 = sq2 + sq1[idx2] - 2.0 * (y * g2).sum(-1)
    return dist1.astype(np.float32), dist2.astype(np.float32)


def kernel(xyz1, xyz2, trace=False, _return_res=False):
    xyz1 = np.asarray(xyz1)
    xyz2 = np.asarray(xyz2)
    b, n, _ = xyz1.shape
    m = xyz2.shape[1]
    key = (n, m, b)
    if key not in _cache:
        _cache[key] = build(n=n, m=m, n_cores=b)
    nc = _cache[key]
    res = _run(nc, xyz1, xyz2, b, trace=trace)

    idx1 = np.stack([r["idx1"].T.reshape(-1) for r in res.results]).astype(np.int32)
    idx2 = np.stack([r["idx2"].reshape(-1) for r in res.results]).astype(np.int32)
    d1l, d2l = [], []
    for bb in range(b):
        d1, d2 = _host_dists(xyz1[bb].astype(np.float32), xyz2[bb].astype(np.float32),
                             idx1[bb], idx2[bb])
        d1l.append(d1)
        d2l.append(d2)
    out = (np.stack(d1l), np.stack(d2l), idx1, idx2)
    if _return_res:
        return out, res
    return out


if __name__ == "__main__":
    rng = np.random.default_rng(0)
    x = rng.standard_normal((8, N, 3), dtype=np.float32)
    y = rng.standard_normal((8, M, 3), dtype=np.float32)
    d1, d2, i1, i2 = kernel(x, y)
    print("ok", d1.shape, d2.shape, i1.shape, i2.shape)
